# revision 1
# baseline (speedup 1.0000x reference)
"""NT-Xent contrastive loss on 8 Trainium2 NeuronCores.

Math (reference): z = l2-normalize rows of concat(emb_i, emb_j) -> [8192, 512].
sim = (z @ z.T) / T with T = 0.5.  denom_r = sum_j exp(sim_rj) - exp(sim_rr),
sim_rr = 1/T exactly, so subtract e^2.  pos pair sim[k, k+N] = 2*cos_k.
loss = (sum_r log(denom_r) - 4 * sum_k cos_k) / 8192.

Sharding: data-parallel over rows of sim.  Each core computes a 1024-row
block of sim against all 8192 columns, reduces to one partial scalar, plus
a 512-pair slice of the positive-pair cosines.  Host sums the 8 partials.

Device pipeline per core (identical SPMD program, per-core data):
  - stream repsT [512, 8192] f32 (host-transposed) in [128, 2048] tiles
    (8KB DMA bursts per partition line)
  - column sums of squares via ones[128,128]-matmul of squares: the PSUM
    result is REPLICATED across all 128 partitions, so rinv =
    exp(-0.5*ln(ss)) runs at full 128-lane ACT rate straight out of PSUM
    and the Exp output IS the per-column scale tile (no partition
    broadcast, no 1-lane row ops)
  - column scale + bf16 cast in one DVE pass: zT = st_f32 * B -> bf16;
    zT lives in a rotating per-group pool (each 2048-column group is
    consumed by exactly one matmul group)
  - all ACT functions used (Square/Ln/Exp/Copy) are pinned to the single
    natural_log_exp_and_others table set -> one ACT_TABLE_LOAD total
  - main matmul: lhsT = own 1024 normalized cols, rhs = all 8192 cols,
    K=512 over 4 chunks, PSUM groups [128, 2048], bf16
  - ACT exp(2*x) with accum_out -> row sums, ln(denom - e^2), reduce
  - emission is software-pipelined two groups ahead so the strict PE
    FIFO never interleaves a group's prep matmuls behind the mains that
    must overlap them
"""

import functools
import math

import numpy as np

import concourse.bacc as bacc
import concourse.bass as bass
import concourse.tile as tile
from concourse import mybir
from concourse.bass_utils import run_bass_kernel_spmd
from concourse.hw_specs import get_activation_tables as _orig_gat

F32 = mybir.dt.float32
BF16 = mybir.dt.bfloat16
AF = mybir.ActivationFunctionType
ALU = mybir.AluOpType

N_CORES = 8
N = 4096              # rows per input
D = 512               # embedding dim
M = 2 * N             # 8192 rows of sim
ROWS_PER_CORE = M // N_CORES      # 1024
POS_PER_CORE = N // N_CORES       # 512
D_CH = D // 128       # 4 contraction chunks
E2 = float(math.exp(2.0))
INV_T = 2.0           # 1 / temperature
GW = 2048             # column-group width

_ONE_SET = "natural_log_exp_and_others"


@functools.cache
def _patched_gat(arch):
    """Pin every ACT function this kernel uses to one table set so the
    table-load chooser emits a single ACT_TABLE_LOAD (the default
    first-match policy alternates sets on every Ln<->Exp transition,
    costing ~2.7us per switch)."""
    t = dict(_orig_gat(arch))
    if _ONE_SET not in t:
        return t
    mine = {AF.Exp, AF.Ln, AF.Square, AF.Copy, AF.Identity}
    return {
        name: (s if name == _ONE_SET else (set(s) - mine))
        for name, s in t.items()
    }


def build_program():
    bacc.get_activation_tables = _patched_gat

    nc = bacc.Bacc(
        "TRN2",
        target_bir_lowering=False,
        debug=False,
        num_devices=N_CORES,
    )

    repsT = nc.dram_tensor("repsT", [D, M], F32, kind="ExternalInput")
    myT = nc.dram_tensor("myT", [D, ROWS_PER_CORE], F32, kind="ExternalInput")
    pi = nc.dram_tensor("pi", [POS_PER_CORE, D], F32, kind="ExternalInput")
    pj = nc.dram_tensor("pj", [POS_PER_CORE, D], F32, kind="ExternalInput")
    out_d = nc.dram_tensor("out", [2, 1], F32, kind="ExternalOutput")

    with tile.TileContext(nc) as tc:
        import contextlib

        with contextlib.ExitStack() as ctx:
            const = ctx.enter_context(tc.tile_pool(name="const", bufs=1))
            big = ctx.enter_context(tc.tile_pool(name="big", bufs=1))
            stage = ctx.enter_context(tc.tile_pool(name="stage", bufs=9))
            sqp = ctx.enter_context(tc.tile_pool(name="sqp", bufs=3))
            bpool = ctx.enter_context(tc.tile_pool(name="bpool", bufs=3))
            lnp = ctx.enter_context(tc.tile_pool(name="lnp", bufs=6))
            ztp = ctx.enter_context(tc.tile_pool(name="ztp", bufs=2))
            posp = ctx.enter_context(tc.tile_pool(name="posp", bufs=4))
            sink = ctx.enter_context(tc.tile_pool(name="sink", bufs=2))
            esink = ctx.enter_context(tc.tile_pool(name="esink", bufs=2))

            ones128 = const.tile([128, 128], BF16)
            nc.vector.memset(ones128[:], 1.0)
            ones_f = const.tile([128, 1], F32)
            nc.vector.memset(ones_f[:], 1.0)
            neg_e2 = const.tile([128, 1], F32)
            nc.vector.memset(neg_e2[:], -E2)

            lhsT = [big.tile([128, ROWS_PER_CORE], BF16, tag=f"lhsT{d}",
                             name=f"lhsT{d}") for d in range(D_CH)]
            dacc = big.tile([128, 32], F32, tag="dacc")
            pos_ssi = big.tile([128, 4], F32, tag="pos_ssi")
            pos_ssj = big.tile([128, 4], F32, tag="pos_ssj")
            pos_dot = big.tile([128, 4], F32, tag="pos_dot")

            pp_main = ctx.enter_context(
                tc.tile_pool(name="pp_main", bufs=2, space="PSUM")
            )

            def emit_prep_group(src, col0, w, dst, label):
                """Normalize w columns of src starting at col0 into dst
                (4 chunk tiles [128, w] bf16).  w in {1024, 2048}."""
                nk = w // 512
                pt = pp_main.tile([128, GW], F32, tag="pp_main",
                                  name=f"ssg_{label}")
                sts = []
                for d in range(D_CH):
                    st = stage.tile([128, GW], F32, tag="stage",
                                    name=f"st_{label}_{d}")
                    nc.sync.dma_start(
                        st[0:128, 0:w], src[bass.ts(d, 128), col0 : col0 + w]
                    )
                    sts.append(st)
                    sqt = sqp.tile([128, GW], BF16, tag="sqp",
                                   name=f"sq_{label}_{d}")
                    if d < 2:
                        nc.scalar.activation(sqt[0:128, 0:w], st[0:128, 0:w],
                                             AF.Square)
                    else:
                        nc.vector.tensor_mul(sqt[0:128, 0:w], st[0:128, 0:w],
                                             st[0:128, 0:w])
                    for k in range(nk):
                        nc.tensor.matmul(
                            pt[:, bass.ts(k, 512)],
                            ones128[:], sqt[:, bass.ts(k, 512)],
                            start=(d == 0), stop=(d == D_CH - 1),
                        )
                bt = bpool.tile([128, GW], BF16, tag="bpool",
                                name=f"B_{label}")
                for k in range(nk):
                    lt = lnp.tile([128, 512], F32, tag="lnp")
                    nc.scalar.activation(lt[:], pt[:, bass.ts(k, 512)], AF.Ln)
                    nc.scalar.activation(bt[:, bass.ts(k, 512)], lt[:],
                                         AF.Exp, scale=-0.5)
                for d in range(D_CH):
                    nc.vector.tensor_mul(
                        dst[d][0:128, 0:w], sts[d][0:128, 0:w],
                        bt[0:128, 0:w],
                    )

            def new_zgroup(jg):
                return [ztp.tile([128, GW], BF16, tag=f"zt{d}",
                                 name=f"zt_{jg}_{d}") for d in range(D_CH)]

            def emit_mains(jg, zg):
                for i in range(8):
                    pt = pp_main.tile([128, GW], F32, tag="pp_main",
                                      name=f"mm_{jg}_{i}")
                    for d in range(D_CH):
                        for jj in range(4):
                            nc.tensor.matmul(
                                pt[:, bass.ts(jj, 512)],
                                lhsT[d][:, bass.ts(i, 128)],
                                zg[d][:, bass.ts(jj, 512)],
                                start=(d == 0), stop=(d == D_CH - 1),
                            )
                    es = esink.tile([128, GW], BF16, tag="esink")
                    k = i * 4 + jg
                    nc.scalar.activation(
                        es[:], pt[:], AF.Exp, scale=INV_T,
                        accum_out=dacc[:, k : k + 1],
                    )

            def emit_pos():
                for t in range(4):
                    pit = posp.tile([128, D], F32, tag="posp")
                    nc.sync.dma_start(pit[:], pi[bass.ts(t, 128), :])
                    pjt = posp.tile([128, D], F32, tag="posp")
                    nc.sync.dma_start(pjt[:], pj[bass.ts(t, 128), :])
                    for src0, src1, acc in (
                        (pit, pit, pos_ssi),
                        (pjt, pjt, pos_ssj),
                        (pit, pjt, pos_dot),
                    ):
                        snk = sink.tile([128, D], F32, tag="sink")
                        nc.vector.tensor_mul(snk[:], src0[:], src1[:])
                        nc.vector.tensor_reduce(
                            acc[:, t : t + 1], snk[:],
                            axis=mybir.AxisListType.X, op=ALU.add,
                        )
                lssi = big.tile([128, 4], F32, tag="lssi")
                lssj = big.tile([128, 4], F32, tag="lssj")
                nc.scalar.activation(lssi[:], pos_ssi[:], AF.Ln)
                nc.scalar.activation(lssj[:], pos_ssj[:], AF.Ln)
                lsum = big.tile([128, 4], F32, tag="lsum")
                nc.vector.tensor_add(lsum[:], lssi[:], lssj[:])
                rinv_ij = big.tile([128, 4], F32, tag="rinv_ij")
                nc.scalar.activation(rinv_ij[:], lsum[:], AF.Exp, scale=-0.5)
                posk = big.tile([128, 4], F32, tag="posk")
                nc.vector.tensor_mul(posk[:], pos_dot[:], rinv_ij[:])
                return posk

            # ------- software-pipelined schedule ----------------------------
            emit_prep_group(myT, 0, ROWS_PER_CORE, lhsT, "my")
            zg = {}
            zg[0] = new_zgroup(0)
            emit_prep_group(repsT, 0, GW, zg[0], "g0")
            zg[1] = new_zgroup(1)
            emit_prep_group(repsT, GW, GW, zg[1], "g1")
            emit_mains(0, zg[0])
            zg[2] = new_zgroup(2)
            emit_prep_group(repsT, 2 * GW, GW, zg[2], "g2")
            posk = emit_pos()
            emit_mains(1, zg[1])
            zg[3] = new_zgroup(3)
            emit_prep_group(repsT, 3 * GW, GW, zg[3], "g3")
            emit_mains(2, zg[2])
            emit_mains(3, zg[3])

            # ------- final reduction ----------------------------------------
            dn = big.tile([128, 8], F32, tag="dn")
            nc.vector.tensor_reduce(
                dn[:], dacc[:].rearrange("p (i g) -> p i g", g=4),
                axis=mybir.AxisListType.X, op=ALU.add,
            )
            ld = big.tile([128, 8], F32, tag="ld")
            nc.scalar.activation(ld[:], dn[:], AF.Ln, bias=neg_e2[:])
            fin = big.tile([128, 2], F32, tag="fin")
            nc.vector.tensor_reduce(
                fin[:, 0:1], ld[:], axis=mybir.AxisListType.X, op=ALU.add
            )
            nc.vector.tensor_reduce(
                fin[:, 1:2], posk[:], axis=mybir.AxisListType.X, op=ALU.add
            )
            fmm = pp_main.tile([128, GW], F32, tag="pp_main", name="fmm")
            nc.tensor.matmul(fmm[0:2, 0:1], fin[:], ones_f[:], start=True,
                             stop=True)
            outsb = big.tile([2, 1], F32, tag="outsb")
            nc.vector.tensor_copy(outsb[:], fmm[0:2, 0:1])
            nc.sync.dma_start(out_d[:], outsb[:])

    nc.compile()
    return nc


_NC_CACHE = None


def _get_program():
    global _NC_CACHE
    if _NC_CACHE is None:
        _NC_CACHE = build_program()
    return _NC_CACHE


def make_in_maps(emb_i: np.ndarray, emb_j: np.ndarray):
    emb_i = np.asarray(emb_i, dtype=np.float32)
    emb_j = np.asarray(emb_j, dtype=np.float32)
    reps = np.concatenate([emb_i, emb_j], axis=0)          # [8192, 512]
    repsT = np.ascontiguousarray(reps.T)                   # [512, 8192]
    in_maps = []
    for c in range(N_CORES):
        in_maps.append(
            {
                "repsT": repsT,
                "myT": np.ascontiguousarray(
                    repsT[:, c * ROWS_PER_CORE : (c + 1) * ROWS_PER_CORE]
                ),
                "pi": np.ascontiguousarray(
                    emb_i[c * POS_PER_CORE : (c + 1) * POS_PER_CORE]
                ),
                "pj": np.ascontiguousarray(
                    emb_j[c * POS_PER_CORE : (c + 1) * POS_PER_CORE]
                ),
            }
        )
    return in_maps


def combine_outputs(results):
    ld_sum = 0.0
    cos_sum = 0.0
    for r in results:
        o = np.asarray(r["out"], dtype=np.float64).reshape(-1)
        ld_sum += o[0]
        cos_sum += o[1]
    loss = (ld_sum - 2.0 * INV_T * cos_sum) / float(M)
    return np.float32(loss)


def kernel(emb_i: np.ndarray, emb_j: np.ndarray) -> np.ndarray:
    nc = _get_program()
    in_maps = make_in_maps(emb_i, emb_j)
    res = run_bass_kernel_spmd(nc, in_maps, list(range(N_CORES)))
    return combine_outputs(res.results)



# revision 19
# speedup vs baseline: 1.4533x; 1.4533x over previous
"""NT-Xent contrastive loss on 8 Trainium2 NeuronCores (symmetric fp8 v3).

Math: z = l2-normalize rows of concat(emb_i, emb_j) -> [8192, 512].
sim = (z @ z.T)/T, T=0.5.  denom_r = sum_j exp(sim_rj) - e^2.
loss = (sum_r ln(denom_r) - 4*sum_k cos_k) / 8192.

sim is symmetric: only the upper triangle of the 16x16 grid of 512-row
strips is computed.  exp(sim) block (r,c) contributes its row sums to
denom[strip r] and its column sums to denom[strip c].  Round-robin
pairing makes the program uniform across cores: core k receives repsT
with columns rotated left by 512k, owns LOCAL row strips 0 and 8, and
computes strip 0 x local cstrips 0..8 plus strip 8 x local cstrips
8..15.  Over k=0..7 this covers each unordered strip pair exactly once.
Per-core partial row/col sums of exp go back to the host, which
assembles denom, takes float64 log, and forms the loss.

Device pipeline per core:
  - repsT arrives bf16 (host cast); per 1024-col group: DVE squares ->
    fp8, ones-weights DoubleRow matmul -> column sums-of-squares
    (PSUM, replicated over partitions), ACT ln then exp(-.5*ln+ln 16)
    -> B = 16/||col|| (bf16), DVE scale-mul -> z tiles (fp8, x16 to
    stay clear of fp8 denormals; exp scale compensates by 1/256)
  - mains: fp8 DoubleRow matmuls (K=512 as 2 passes of 2x128), PSUM
    [128,1024] groups, ACT exp(scale=2/256) -> es bf16 + accum_out row
    sums; colsum: ones-bf16 matmul chains over the 4 row tiles of each
    off-diag 512-block -> PSUM -> DMA one partition row to DRAM
  - positive pairs: separate bf16 row-major path (DVE fused
    multiply-reduce), cos_k per pair -> DRAM
  - ACT functions (Exp/Ln/Copy) pinned to one table set -> single
    ACT_TABLE_LOAD
"""

import functools
import math
import os

import numpy as np
import ml_dtypes

import concourse.bacc as bacc
import concourse.bass as bass
import concourse.tile as tile
from concourse import mybir
from concourse.bass_utils import run_bass_kernel_spmd
from concourse.hw_specs import get_activation_tables as _orig_gat

F32 = mybir.dt.float32
BF16 = mybir.dt.bfloat16
FP8 = mybir.dt.float8e4
AF = mybir.ActivationFunctionType
ALU = mybir.AluOpType
DR = mybir.MatmulPerfMode.DoubleRow

N_CORES = 8
N = 4096              # rows per input
D = 512               # embedding dim
M = 2 * N             # 8192 rows of sim
NSTRIP = 16           # 512-row strips
SW = 512              # strip width
GW = 1024             # column group width (PSUM group size)
POS_PER_CORE = N // N_CORES       # 512
E2 = float(math.exp(2.0))
INV_T = 2.0           # 1 / temperature
ZSCALE = 16.0         # fp8 z pre-scale (avoids fp8 denormals)
EXP_SCALE = INV_T / (ZSCALE * ZSCALE)

_ONE_SET = "natural_log_exp_and_others"

# mains subgroups, uniform for every core (local indices):
#   (strip_sel, group, col_off, width); strip A = local strip 0
#   (lhsT = group 0 cols [0,512)), strip B = local strip 8 (lhsT =
#   group 4 cols [0,512)).
SUBS = (
    (0, 0, 0, 1024),
    (0, 1, 0, 1024),
    (0, 2, 0, 1024),
    (0, 3, 0, 1024),
    (0, 4, 0, 512),
    (1, 4, 0, 1024),
    (1, 5, 0, 1024),
    (1, 6, 0, 1024),
    (1, 7, 0, 1024),
)
_DIAG_CSUB = {0: 0, 1: 8}  # strip_sel -> local diag cstrip


def _sub_csubs(sub):
    """Local 512-col strips covered by a mains subgroup, with the
    diagonal one excluded (those need no colsum)."""
    s, g, off, w = sub
    c0 = (g * GW + off) // SW
    return [c for c in range(c0, c0 + w // SW) if c != _DIAG_CSUB[s]]


COLSUM_LIST = [(si, c) for si, sub in enumerate(SUBS) for c in _sub_csubs(sub)]
assert len(COLSUM_LIST) == 15


@functools.cache
def _patched_gat(arch):
    """Pin every ACT function this kernel uses to one table set so the
    table-load chooser emits a single ACT_TABLE_LOAD."""
    t = dict(_orig_gat(arch))
    if _ONE_SET not in t:
        return t
    mine = {AF.Exp, AF.Ln, AF.Square, AF.Copy, AF.Identity}
    return {
        name: (s if name == _ONE_SET else (set(s) - mine))
        for name, s in t.items()
    }


USE_BF16 = os.environ.get("K_BF16", "") != ""         # bf16 instead of fp8
USE_DR = os.environ.get("K_NO_DR", "") == "" and not USE_BF16
USE_COLSUMS = os.environ.get("K_NO_COLSUMS", "") == ""
USE_POS = os.environ.get("K_NO_POS", "") == ""
ZDT = BF16 if USE_BF16 else FP8


def build_program():
    bacc.get_activation_tables = _patched_gat

    nc = bacc.Bacc(
        "TRN2",
        target_bir_lowering=False,
        debug=False,
        num_devices=N_CORES,
    )

    repsT = nc.dram_tensor("repsT", [D, M], BF16, kind="ExternalInput")
    pi = nc.dram_tensor("pi", [POS_PER_CORE, D], BF16, kind="ExternalInput")
    pj = nc.dram_tensor("pj", [POS_PER_CORE, D], BF16, kind="ExternalInput")
    out_row = nc.dram_tensor("out_row", [128, 64], F32, kind="ExternalOutput")
    out_col = nc.dram_tensor("out_col", [1, 15 * SW], F32, kind="ExternalOutput")
    out_pos = nc.dram_tensor("out_pos", [128, 4], F32, kind="ExternalOutput")

    with tile.TileContext(nc) as tc:
        import contextlib

        with contextlib.ExitStack() as ctx:
            const = ctx.enter_context(tc.tile_pool(name="const", bufs=1))
            big = ctx.enter_context(tc.tile_pool(name="big", bufs=1))
            stage = ctx.enter_context(tc.tile_pool(name="stage", bufs=8))
            sqp = ctx.enter_context(tc.tile_pool(name="sqp", bufs=4))
            lnpool = ctx.enter_context(tc.tile_pool(name="lnpool", bufs=2))
            bpool = ctx.enter_context(tc.tile_pool(name="bpool", bufs=3))
            esp = ctx.enter_context(tc.tile_pool(name="esp", bufs=8))
            posp = ctx.enter_context(tc.tile_pool(name="posp", bufs=4))
            sink = ctx.enter_context(tc.tile_pool(name="sink", bufs=2))

            ones_bf = const.tile([128, 128], BF16)
            nc.vector.memset(ones_bf[:], 1.0)
            ones_dr = const.tile([128, 2, 128], ZDT)
            nc.vector.memset(ones_dr[:], 1.0)
            ln_zs = const.tile([128, 1], F32)
            nc.vector.memset(ln_zs[:], float(math.log(ZSCALE)))

            # resident z tiles: per 1024-group, two chunk-pair tiles
            # [128, 2, GW] fp8 (pair A = K rows 0..255, pair B = 256..511)
            zq = [
                [big.tile([128, 2, GW], ZDT, tag=f"zq{g}{p}",
                          name=f"zq{g}{p}") for p in range(2)]
                for g in range(8)
            ]
            dacc = big.tile([128, 64], F32, tag="dacc")
            nc.vector.memset(dacc[:], 0.0)
            colrow = big.tile([1, 15 * SW], F32, tag="colrow")
            pos_ssi = big.tile([128, 4], F32, tag="pos_ssi")
            pos_ssj = big.tile([128, 4], F32, tag="pos_ssj")
            pos_dot = big.tile([128, 4], F32, tag="pos_dot")

            pp = ctx.enter_context(
                tc.tile_pool(name="pp", bufs=2, space="PSUM")
            )
            pc = ctx.enter_context(
                tc.tile_pool(name="pc", bufs=4, space="PSUM")
            )

            def prep(g):
                pt = pp.tile([128, GW], F32, tag="pp", name=f"pt{g}")
                sq = [sqp.tile([128, 2, GW], ZDT, tag="sqp",
                               name=f"sq{g}{p}") for p in range(2)]
                sts = []
                for d in range(4):
                    st = stage.tile([128, GW], BF16, tag="stage",
                                    name=f"st{g}{d}")
                    nc.sync.dma_start(
                        st[:], repsT[bass.ts(d, 128), bass.ts(g, GW)]
                    )
                    sts.append(st)
                    nc.vector.tensor_mul(sq[d // 2][:, d % 2, :], st[:], st[:])
                if USE_DR:
                    for p in range(2):
                        for jj in range(2):
                            nc.tensor.matmul(
                                pt[:, bass.ts(jj, 512)],
                                ones_dr[:],
                                sq[p][:, :, bass.ts(jj, 512)],
                                start=(p == 0), stop=(p == 1),
                                perf_mode=DR,
                            )
                else:
                    for p in range(2):
                        for q in range(2):
                            for jj in range(2):
                                nc.tensor.matmul(
                                    pt[:, bass.ts(jj, 512)],
                                    ones_dr[:, 0, :],
                                    sq[p][:, q, bass.ts(jj, 512)],
                                    start=(p == 0 and q == 0),
                                    stop=(p == 1 and q == 1),
                                )
                lt = lnpool.tile([128, GW], BF16, tag="lnpool", name=f"lt{g}")
                nc.scalar.activation(lt[:], pt[:], AF.Ln)
                bt = bpool.tile([128, GW], BF16, tag="bpool", name=f"B{g}")
                nc.scalar.activation(bt[:], lt[:], AF.Exp, scale=-0.5,
                                     bias=ln_zs[:])
                for d in range(4):
                    nc.vector.tensor_mul(
                        zq[g][d // 2][:, d % 2, :], sts[d][:], bt[:]
                    )

            es_tiles = {}

            def mains(si):
                s, g, off, w = SUBS[si]
                lg = 0 if s == 0 else 4  # lhsT group (local cstrip 0 / 8)
                for i in range(4):
                    pm = pp.tile([128, GW], F32, tag="pp",
                                 name=f"pm{si}_{i}")
                    if USE_DR:
                        for p in range(2):
                            for jj in range(w // 512):
                                nc.tensor.matmul(
                                    pm[:, bass.ts(jj, 512)],
                                    zq[lg][p][:, :, bass.ts(i, 128)],
                                    zq[g][p][:, :, off + jj * 512:
                                             off + jj * 512 + 512],
                                    start=(p == 0), stop=(p == 1),
                                    perf_mode=DR,
                                )
                    else:
                        for p in range(2):
                            for q in range(2):
                                for jj in range(w // 512):
                                    nc.tensor.matmul(
                                        pm[:, bass.ts(jj, 512)],
                                        zq[lg][p][:, q, bass.ts(i, 128)],
                                        zq[g][p][:, q, off + jj * 512:
                                                 off + jj * 512 + 512],
                                        start=(p == 0 and q == 0),
                                        stop=(p == 1 and q == 1),
                                    )
                    es = esp.tile([128, GW], BF16, tag="esp",
                                  name=f"es{si}_{i}")
                    slot = s * 32 + i * 8 + g
                    nc.scalar.activation(
                        es[:, 0:w], pm[:, 0:w], AF.Exp, scale=EXP_SCALE,
                        accum_out=dacc[:, slot:slot + 1],
                    )
                    es_tiles[(si, i)] = es

            col_j = 0

            def colsums(si):
                nonlocal col_j
                if not USE_COLSUMS:
                    return
                s, g, off, w = SUBS[si]
                c0 = (g * GW + off) // SW
                for c in _sub_csubs(SUBS[si]):
                    eoff = (c - c0) * SW
                    pcT = pc.tile([128, SW], F32, tag="pc",
                                  name=f"pc{si}_{c}")
                    for i in range(4):
                        nc.tensor.matmul(
                            pcT[:],
                            ones_bf[:],
                            es_tiles[(si, i)][:, eoff:eoff + SW],
                            start=(i == 0), stop=(i == 3),
                        )
                    nc.vector.tensor_copy(
                        colrow[0:1, col_j * SW:(col_j + 1) * SW], pcT[0:1, :]
                    )
                    col_j += 1

            def pos_products():
                for t in range(4):
                    pit = posp.tile([128, D], BF16, tag="posp")
                    nc.sync.dma_start(pit[:], pi[bass.ts(t, 128), :])
                    pjt = posp.tile([128, D], BF16, tag="posp")
                    nc.sync.dma_start(pjt[:], pj[bass.ts(t, 128), :])
                    for src0, src1, acc in (
                        (pit, pit, pos_ssi),
                        (pjt, pjt, pos_ssj),
                        (pit, pjt, pos_dot),
                    ):
                        snk = sink.tile([128, D], BF16, tag="sink")
                        nc.vector.tensor_mul(snk[:], src0[:], src1[:])
                        nc.vector.tensor_reduce(
                            acc[:, t:t + 1], snk[:],
                            axis=mybir.AxisListType.X, op=ALU.add,
                        )

            def pos_finish():
                lssi = big.tile([128, 4], F32, tag="lssi")
                lssj = big.tile([128, 4], F32, tag="lssj")
                nc.scalar.activation(lssi[:], pos_ssi[:], AF.Ln)
                nc.scalar.activation(lssj[:], pos_ssj[:], AF.Ln)
                lsum = big.tile([128, 4], F32, tag="lsum")
                nc.vector.tensor_add(lsum[:], lssi[:], lssj[:])
                rinv_ij = big.tile([128, 4], F32, tag="rinv_ij")
                nc.scalar.activation(rinv_ij[:], lsum[:], AF.Exp, scale=-0.5)
                posk = big.tile([128, 4], F32, tag="posk")
                nc.vector.tensor_mul(posk[:], pos_dot[:], rinv_ij[:])
                nc.sync.dma_start(out_pos[:], posk[:])

            # ------- software-pipelined schedule -----------------------
            prep(0)
            prep(1)
            if USE_POS:
                pos_products()
            mains(0)
            prep(2)
            mains(1)
            colsums(0)
            prep(3)
            mains(2)
            colsums(1)
            prep(4)
            mains(3)
            colsums(2)
            prep(5)
            mains(4)
            colsums(3)
            prep(6)
            mains(5)
            colsums(4)
            prep(7)
            mains(6)
            colsums(5)
            mains(7)
            colsums(6)
            mains(8)
            colsums(7)
            colsums(8)
            if USE_POS:
                pos_finish()
            else:
                posk = big.tile([128, 4], F32, tag="posk")
                nc.vector.memset(posk[:], 0.0)
                nc.sync.dma_start(out_pos[:], posk[:])
            nc.sync.dma_start(out_row[:], dacc[:])
            if USE_COLSUMS:
                nc.sync.dma_start(out_col[:], colrow[:])
            else:
                nc.vector.memset(colrow[:], 0.0)
                nc.sync.dma_start(out_col[:], colrow[:])

    nc.compile()
    return nc


_NC_CACHE = None


def _get_program():
    global _NC_CACHE
    if _NC_CACHE is None:
        _NC_CACHE = build_program()
    return _NC_CACHE


def make_in_maps(emb_i: np.ndarray, emb_j: np.ndarray):
    emb_i = np.asarray(emb_i, dtype=np.float32)
    emb_j = np.asarray(emb_j, dtype=np.float32)
    reps = np.concatenate([emb_i, emb_j], axis=0)          # [8192, 512]
    repsT = np.ascontiguousarray(reps.T).astype(ml_dtypes.bfloat16)
    in_maps = []
    for c in range(N_CORES):
        in_maps.append(
            {
                "repsT": np.ascontiguousarray(
                    np.roll(repsT, -SW * c, axis=1)
                ),
                "pi": emb_i[c * POS_PER_CORE:(c + 1) * POS_PER_CORE].astype(
                    ml_dtypes.bfloat16
                ),
                "pj": emb_j[c * POS_PER_CORE:(c + 1) * POS_PER_CORE].astype(
                    ml_dtypes.bfloat16
                ),
            }
        )
    return in_maps


def combine_outputs(results):
    """Assemble denom from per-core partial row/col sums, then the loss."""
    rs = np.zeros(M, dtype=np.float64)
    cos_sum = 0.0
    for k, r in enumerate(results):
        dacc = np.asarray(r["out_row"], dtype=np.float64)    # [128, 64]
        for s, rstrip in enumerate((k, k + 8)):
            base = rstrip * SW
            for i in range(4):
                sl = dacc[:, s * 32 + i * 8:s * 32 + i * 8 + 8].sum(axis=1)
                rs[base + 128 * i: base + 128 * i + 128] += sl
        ocol = np.asarray(r["out_col"], dtype=np.float64).reshape(15, SW)
        for j, (si, c_loc) in enumerate(COLSUM_LIST):
            c_glob = (k + c_loc) % NSTRIP
            rs[c_glob * SW:(c_glob + 1) * SW] += ocol[j]
        cos_sum += float(np.asarray(r["out_pos"], dtype=np.float64).sum())
    denom = rs - E2
    loss = (np.log(denom).sum() - 2.0 * INV_T * cos_sum) / float(M)
    return np.float32(loss)


def kernel(emb_i: np.ndarray, emb_j: np.ndarray) -> np.ndarray:
    nc = _get_program()
    in_maps = make_in_maps(emb_i, emb_j)
    res = run_bass_kernel_spmd(nc, in_maps, list(range(N_CORES)))
    return combine_outputs(res.results)
